# revision 4
# baseline (speedup 1.0000x reference)
"""Hadamard transform kernel for Trainium2 (8 NeuronCores, SPMD).

Problem: x (8192, 4096) fp32; apply a 128-point Hadamard transform to each
contiguous 128-element group of every row.  Equivalent to
    out = (x.reshape(-1, 128) @ M).reshape(8192, 4096)
where M is the 128x128 butterfly matrix (symmetric, entries +/- 2^-3.5).

The op is HBM-bandwidth bound, so transport precision is the lever:
  - input:  int8 symmetric quantization (1 B/elem), HWDGE-loaded raw;
    the idle GpSimd engine casts int8 -> fp16 in SBUF (engine ports, so
    neither the HBM nor the DMA/SBUF-AXI budget pays for the expansion);
    the int8 step scale is folded into the Hadamard matrix on the host.
  - output: fp16 (2 B/elem), upcast to fp32 on the host.
End-to-end rel err ~1.4e-2 (dominated by int8 quantization; tol 2e-2).

Layout trick: the host pre-packs each core's row-shard into k-major form
    xT[k, g*1024 + r] = x_core[r, g*128 + k]          (shape [128, 32768])
so every 128-element Hadamard group lies along the partition axis.  The
device then computes a single streaming matmul
    outT = (M*s)^T @ xT        (all groups share M)
with no on-chip transposes: load chunk -> gpsimd cast -> matmul -> PSUM
-> DVE/ACT cast-copy to fp16 -> store.  The host unpacks outT with the
inverse (involutive) permutation and upcasts to fp32.

Per core: 4 MiB in + 8 MiB out at ~358 GB/s/NC HBM => ~35 us floor.
"""

import math

import numpy as np

import concourse.bass as bass
import concourse.tile as tile
from concourse import bacc, mybir
from concourse.bass import ts
from concourse.bass_utils import run_bass_kernel_spmd

N_CORES = 8
ROWS, COLS = 8192, 4096
R_CORE = ROWS // N_CORES  # 1024 rows per core
G = 128                   # hadamard group size
NG = COLS // G            # 32 groups per row
F = R_CORE * NG           # 32768 free-dim elements per core
MM_W = 512                # matmul moving width (one fp32 PSUM bank)
# smaller edge chunks shorten pipeline fill and drain
CHUNKS = [1024] + [2048] * 15 + [1024]

I8 = mybir.dt.int8
F16 = mybir.dt.float16
F32 = mybir.dt.float32


def _hadamard_matrix() -> np.ndarray:
    """M = butterfly(I_128): out_row = x_row @ M (M symmetric)."""
    x = np.eye(G, dtype=np.float64)[..., None]
    for _ in range(int(math.log2(G))):
        top = x[..., ::2, :] + x[..., 1::2, :]
        bot = x[..., ::2, :] - x[..., 1::2, :]
        x = np.concatenate((top, bot), axis=-1) * (0.5 ** 0.5)
    return np.ascontiguousarray(x.squeeze(-2))


def _build_module():
    nc = bacc.Bacc("TRN2", target_bir_lowering=False, debug=False)
    x_d = nc.dram_tensor("x", [G, F], I8, kind="ExternalInput")
    h_d = nc.dram_tensor("hmat", [G, G], F16, kind="ExternalInput")
    o_d = nc.dram_tensor("out", [G, F], F16, kind="ExternalOutput")

    with tile.TileContext(nc) as tc:
        with (
            tc.tile_pool(name="const", bufs=1) as cpool,
            tc.tile_pool(name="xq", bufs=6) as qpool,
            tc.tile_pool(name="xin", bufs=6) as xpool,
            tc.tile_pool(name="outb", bufs=6) as opool,
            tc.tile_pool(name="ps", bufs=4, space=bass.MemorySpace.PSUM) as ps,
        ):
            # hmat + ACT-table prime ride the Scalar ring (idle until the
            # first store); x loads get the Sync ring to themselves.
            hm = cpool.tile([G, G], F16)
            nc.scalar.dma_start(hm[:], h_d[:])
            wsb = cpool.tile([G, G], F16)
            nc.gpsimd.memset(wsb[:], 1.0)
            nc.scalar.copy(wsb[:, 0:1], wsb[:, 1:2])  # ACT_TABLE_LOAD now
            # PE warmup (HAM clock-gate) during the initial DMA wait.
            for _ in range(16):
                wp = ps.tile([G, 1024], F32, tag="pm")
                nc.tensor.matmul(wp[:, 0:G], wsb[:], wsb[:])

            f0 = 0
            for ci, cw in enumerate(CHUNKS):
                xq = qpool.tile([G, cw], I8, tag="xq")
                nc.sync.dma_start(xq[:], x_d[:, f0:f0 + cw])
                xt = xpool.tile([G, cw], F16, tag="xt")
                nc.gpsimd.tensor_copy(xt[:], xq[:])
                ot = opool.tile([G, cw], F16, tag="ot")
                # 2-bank psum tiles; DVE casts even tiles, ACT odd tiles,
                # so the psum->fp16 copies run on both engines in parallel.
                for p in range(cw // 1024):
                    pm = ps.tile([G, 1024], F32, tag="pm")
                    nc.tensor.matmul(
                        pm[:, 0:MM_W], hm[:], xt[:, ts(2 * p, MM_W)]
                    )
                    nc.tensor.matmul(
                        pm[:, MM_W:1024], hm[:], xt[:, ts(2 * p + 1, MM_W)]
                    )
                    dst = ot[:, p * 1024:(p + 1) * 1024]
                    if (ci + p) % 2 == 0:
                        nc.vector.tensor_copy(dst, pm[:])
                    else:
                        nc.scalar.copy(dst, pm[:])
                nc.scalar.dma_start(o_d[:, f0:f0 + cw], ot[:])
                f0 += cw

    nc.compile()
    return nc


_NC_CACHE = None


def _get_module():
    global _NC_CACHE
    if _NC_CACHE is None:
        _NC_CACHE = _build_module()
    return _NC_CACHE


def _prep_inputs(x: np.ndarray) -> list[dict]:
    """Full fp32 x -> per-core in_maps (int8 quantized, k-major pack)."""
    amax = float(np.abs(x).max())
    step = amax / 127.0 if amax > 0 else 1.0
    xq = np.clip(np.rint(x * (1.0 / step)), -127, 127).astype(np.int8)
    hmat = (_hadamard_matrix() * step).astype(np.float16)
    in_maps = []
    for c in range(N_CORES):
        xc = xq[c * R_CORE:(c + 1) * R_CORE]
        xt = np.ascontiguousarray(
            xc.reshape(R_CORE, NG, G).transpose(2, 1, 0)
        ).reshape(G, F)
        in_maps.append({"x": xt, "hmat": hmat})
    return in_maps


def _postprocess(results) -> np.ndarray:
    outs = []
    for r in results:
        ot = np.asarray(r["out"]).reshape(G, NG, R_CORE).transpose(2, 1, 0)
        outs.append(ot.reshape(R_CORE, COLS).astype(np.float32))
    return np.concatenate(outs, axis=0)


def kernel(x) -> np.ndarray:
    x = np.ascontiguousarray(np.asarray(x, dtype=np.float32))
    assert x.shape == (ROWS, COLS)
    nc = _get_module()
    in_maps = _prep_inputs(x)
    res = run_bass_kernel_spmd(nc, in_maps, core_ids=list(range(N_CORES)))
    return _postprocess(res.results)


# revision 6
# speedup vs baseline: 2.3867x; 2.3867x over previous
"""Hadamard transform kernel for Trainium2 (8 NeuronCores, SPMD).

Problem: x (8192, 4096) fp32; apply a 128-point Hadamard transform to each
contiguous 128-element group of every row.  Equivalent to
    out = (x.reshape(-1, 128) @ M).reshape(8192, 4096)
where M is the 128x128 butterfly matrix (symmetric, entries +/- 2^-3.5).

The op is HBM-bandwidth bound, so transport precision is the lever:
  - input:  int8 symmetric quantization (1 B/elem); the int8 step scale
    is folded into the Hadamard matrix on the host.
  - output: fp16 (2 B/elem), upcast to fp32 on the host.
End-to-end rel err ~1.4e-2 (dominated by int8 quantization; tol 2e-2).

The int8 -> fp16 expansion before the matmul alternates between two
paths so no single resource saturates:
  - even chunks: SWDGE cast-load (GpSimd ring) converts in the DMA
    datapath - no engine time, but 2 B/elem on the SBUF AXI write side;
  - odd chunks:  HWDGE raw int8 load (Sync ring) + DVE cast - 1 B/elem
    on the fabric, ~1.2 us of DVE time per chunk.
PSUM->fp16 copies are split DVE/ACT; stores ride the Scalar ring, which
is kept light so store issue never throttles (v2 lesson: a 1.3 us
ACTIVATE ahead of every store capped stores at ~240 GB/s).

Layout trick: the host pre-packs each core's row-shard into k-major form
    xT[k, g*1024 + r] = x_core[r, g*128 + k]          (shape [128, 32768])
so every 128-element Hadamard group lies along the partition axis.  The
device then computes a single streaming matmul  outT = (M*s)^T @ xT
(all groups share M) with no on-chip transposes.  The host unpacks outT
with the inverse (involutive) permutation and upcasts to fp32.

Per core: 4 MiB in + 8 MiB out at ~358 GB/s/NC HBM => ~35 us floor.
"""

import math

import numpy as np

import concourse.bass as bass
import concourse.tile as tile
from concourse import bacc, mybir
from concourse.bass import ts
from concourse.bass_utils import run_bass_kernel_spmd

N_CORES = 8
ROWS, COLS = 8192, 4096
R_CORE = ROWS // N_CORES  # 1024 rows per core
G = 128                   # hadamard group size
NG = COLS // G            # 32 groups per row
F = R_CORE * NG           # 32768 free-dim elements per core
MM_W = 512                # matmul moving width (one fp32 PSUM bank)
# smaller edge chunks shorten pipeline fill and drain
CHUNKS = [1024] + [2048] * 15 + [1024]

I8 = mybir.dt.int8
F16 = mybir.dt.float16
F32 = mybir.dt.float32


def _hadamard_matrix() -> np.ndarray:
    """M = butterfly(I_128): out_row = x_row @ M (M symmetric)."""
    x = np.eye(G, dtype=np.float64)[..., None]
    for _ in range(int(math.log2(G))):
        top = x[..., ::2, :] + x[..., 1::2, :]
        bot = x[..., ::2, :] - x[..., 1::2, :]
        x = np.concatenate((top, bot), axis=-1) * (0.5 ** 0.5)
    return np.ascontiguousarray(x.squeeze(-2))


def _build_module():
    nc = bacc.Bacc("TRN2", target_bir_lowering=False, debug=False)
    x_d = nc.dram_tensor("x", [G, F], I8, kind="ExternalInput")
    h_d = nc.dram_tensor("hmat", [G, G], F16, kind="ExternalInput")
    o_d = nc.dram_tensor("out", [G, F], F16, kind="ExternalOutput")

    with tile.TileContext(nc) as tc:
        with (
            tc.tile_pool(name="const", bufs=1) as cpool,
            tc.tile_pool(name="xq", bufs=4) as qpool,
            tc.tile_pool(name="xin", bufs=6) as xpool,
            tc.tile_pool(name="outb", bufs=6) as opool,
            tc.tile_pool(name="ps", bufs=4, space=bass.MemorySpace.PSUM) as ps,
        ):
            # hmat + ACT-table prime ride the Scalar ring (idle until the
            # first store); x loads get the Sync/GpSimd rings to themselves.
            hm = cpool.tile([G, G], F16)
            nc.scalar.dma_start(hm[:], h_d[:])
            wsb = cpool.tile([G, G], F16)
            nc.gpsimd.memset(wsb[:], 1.0)
            nc.scalar.copy(wsb[:, 0:1], wsb[:, 1:2])  # ACT_TABLE_LOAD now
            # PE warmup (HAM clock-gate) during the initial DMA wait.
            for _ in range(16):
                wp = ps.tile([G, 1024], F32, tag="pm")
                nc.tensor.matmul(wp[:, 0:G], wsb[:], wsb[:])

            f0 = 0
            for ci, cw in enumerate(CHUNKS):
                swdge = (ci % 2 == 0)
                xt = xpool.tile([G, cw], F16, tag="xt")
                if swdge:
                    # cast int8->fp16 inside the DMA datapath
                    nc.gpsimd.dma_start(xt[:], x_d[:, f0:f0 + cw])
                else:
                    xq = qpool.tile([G, cw], I8, tag="xq")
                    nc.sync.dma_start(xq[:], x_d[:, f0:f0 + cw])
                    nc.vector.tensor_copy(xt[:], xq[:])
                ot = opool.tile([G, cw], F16, tag="ot")
                for p in range(cw // 1024):
                    pm = ps.tile([G, 1024], F32, tag="pm")
                    nc.tensor.matmul(
                        pm[:, 0:MM_W], hm[:], xt[:, ts(2 * p, MM_W)]
                    )
                    nc.tensor.matmul(
                        pm[:, MM_W:1024], hm[:], xt[:, ts(2 * p + 1, MM_W)]
                    )
                    dst = ot[:, p * 1024:(p + 1) * 1024]
                    # split psum copies DVE/ACT; keep both under the
                    # ~2.2us per-chunk HBM period.
                    if p == 0:
                        nc.vector.tensor_copy(dst, pm[:])
                    else:
                        nc.scalar.copy(dst, pm[:])
                nc.scalar.dma_start(o_d[:, f0:f0 + cw], ot[:])
                f0 += cw

    nc.compile()
    return nc


_NC_CACHE = None


def _get_module():
    global _NC_CACHE
    if _NC_CACHE is None:
        _NC_CACHE = _build_module()
    return _NC_CACHE


def _prep_inputs(x: np.ndarray) -> list[dict]:
    """Full fp32 x -> per-core in_maps (int8 quantized, k-major pack)."""
    amax = float(np.abs(x).max())
    step = amax / 127.0 if amax > 0 else 1.0
    xq = np.clip(np.rint(x * (1.0 / step)), -127, 127).astype(np.int8)
    hmat = (_hadamard_matrix() * step).astype(np.float16)
    in_maps = []
    for c in range(N_CORES):
        xc = xq[c * R_CORE:(c + 1) * R_CORE]
        xt = np.ascontiguousarray(
            xc.reshape(R_CORE, NG, G).transpose(2, 1, 0)
        ).reshape(G, F)
        in_maps.append({"x": xt, "hmat": hmat})
    return in_maps


def _postprocess(results) -> np.ndarray:
    outs = []
    for r in results:
        ot = np.asarray(r["out"]).reshape(G, NG, R_CORE).transpose(2, 1, 0)
        outs.append(ot.reshape(R_CORE, COLS).astype(np.float32))
    return np.concatenate(outs, axis=0)


def kernel(x) -> np.ndarray:
    x = np.ascontiguousarray(np.asarray(x, dtype=np.float32))
    assert x.shape == (ROWS, COLS)
    nc = _get_module()
    in_maps = _prep_inputs(x)
    res = run_bass_kernel_spmd(nc, in_maps, core_ids=list(range(N_CORES)))
    return _postprocess(res.results)
